# revision 4
# baseline (speedup 1.0000x reference)
"""Self-contained Trainium2 Bass kernel for deformable conv 2d.

kernel(x, offset, weight) -> out, matching the jax reference:
  x[2,256,64,64] f32, offset[2,18,64,64] f32, weight[256,256,3,3] f32
  -> out[2,256,64,64] f32 (KH=KW=3, stride=1, pad=1, dil=1, DG=1).

Runs SPMD on 8 NeuronCores, data-parallel: core = (batch, spatial quarter).

Device pipeline per tap k (host precomputes gather rows + blend weights):
  1. one batched dma_gather: 1024 rows of the pre-expanded patch image
     xi_ov[4096, 1024] (row r = 2x2 patch [TL, BL, TR, BR] x 256ch bf16)
     -> vt [128 samples, 8 slots, 1024]
  2. bilinear blend split across engines (per slot, per-partition scalars):
     ACT: sA = TL*w0, sB = BL*w1; GPSIMD: sC = TR*w2;
     DVE: tD = BR*w3 + sA (stt), then two [128, 2048] adds -> cr
  3. 16 HWDGE xbar transposes [128s,128c] -> rhsT [128c, 2, 1024s]
  4. PE matmuls accumulate over 18 (tap, cin-chunk) into PSUM [2][128, 1024]
"""

import sys

for _p in ("/opt/trn_rl_repo",):
    if _p not in sys.path:
        sys.path.insert(0, _p)


import numpy as np
import ml_dtypes

import concourse.bass as bass
import concourse.mybir as mybir
import concourse.tile as tile

F32 = mybir.dt.float32
BF16 = mybir.dt.bfloat16
I16 = mybir.dt.int16

N, CIN, H, W = 2, 256, 64, 64
COUT = 256
KH = KW = 3
K = KH * KW
S = H * W            # 4096 positions per batch
SLOC = S // 4        # 1024 per core
TPC = 8              # slots per tap (SLOC/128)
NT = K * TPC         # 72 (slot index j = k*8 + t)

AluOp = mybir.AluOpType


def build_core_kernel(nc, tc, outs, ins):
    """Emit the per-core kernel. ins/outs are dicts of DRAM APs."""
    from contextlib import ExitStack

    xi = ins["xi"]          # [4096, 1024] bf16 patch rows
    wT = ins["wT"]          # [2304, 256] bf16 lhsT
    cwf = ins["cwf"]        # [128, 4, 72] f32 blend weights
    idxs = ins["idxs"]      # [128, 9, 64] int16 gather rows
    out = outs["out"]       # [128, 2, 1024] f32

    ctx = ExitStack()
    sp = ctx.enter_context(tc.tile_pool(name="static", bufs=1))
    vtp = ctx.enter_context(tc.tile_pool(name="vt", bufs=2))
    mp = ctx.enter_context(tc.tile_pool(name="mults", bufs=2))
    crp = ctx.enter_context(tc.tile_pool(name="cr", bufs=2))
    rp = ctx.enter_context(tc.tile_pool(name="rhsT", bufs=2))
    pp = ctx.enter_context(tc.tile_pool(name="psum", bufs=1, space="PSUM"))

    v = nc.vector

    # static loads; idxs first (gates the gathers)
    idxs_s = sp.tile([128, K, 64], I16, name="idxs_s")
    nc.sync.dma_start(idxs_s[:], idxs)
    cwf_s = sp.tile([128, 4, NT], F32, name="cwf_s")
    nc.sync.dma_start(cwf_s[:], cwf)
    wT_s = sp.tile([128, 18, 256], BF16, name="wT_s")
    nc.sync.dma_start(wT_s[:], wT.rearrange("(j p) o -> p j o", p=128))

    ps = [pp.tile([128, SLOC], F32, name=f"psum{h}") for h in range(2)]
    osb = sp.tile([128, 2, SLOC], F32, name="osb")

    def gather(k):
        vt = vtp.tile([128, TPC, 1024], BF16, name="vt")
        nc.gpsimd.dma_gather(
            out_ap=vt[:],
            in_ap=xi,
            idxs_ap=idxs_s[:, k, :],
            num_idxs=SLOC,
            num_idxs_reg=SLOC,
            elem_size=1024,
        )
        return vt

    def process(k, vt):
        sA = mp.tile([128, TPC, 256], BF16, name="sA")
        sB = mp.tile([128, TPC, 256], BF16, name="sB")
        sC = mp.tile([128, TPC, 256], BF16, name="sC")
        tD = crp.tile([128, TPC, 256], BF16, name="tD")
        u = crp.tile([128, TPC, 256], BF16, name="u")
        cr = crp.tile([128, TPC, 256], BF16, name="cr")

        for t in range(TPC):
            j = k * TPC + t
            nc.scalar.mul(sA[:, t], vt[:, t, 0:256], cwf_s[:, 0, j : j + 1])
            nc.scalar.mul(sB[:, t], vt[:, t, 256:512], cwf_s[:, 1, j : j + 1])
            nc.gpsimd.tensor_scalar(
                sC[:, t], vt[:, t, 512:768], cwf_s[:, 2, j : j + 1], None, AluOp.mult
            )
        for t in range(TPC):
            j = k * TPC + t
            v.scalar_tensor_tensor(
                tD[:, t], vt[:, t, 768:1024], cwf_s[:, 3, j : j + 1], sA[:, t],
                AluOp.mult, AluOp.add,
            )
        v.tensor_tensor(u[:], tD[:], sB[:], AluOp.add)
        v.tensor_tensor(cr[:], u[:], sC[:], AluOp.add)

        rhsT = rp.tile([128, 2, SLOC], BF16, name="rhsT")
        for t in range(TPC):
            for h in range(2):
                nc.sync.dma_start_transpose(
                    rhsT[:, h, t * 128 : (t + 1) * 128],
                    cr[:, t, h * 128 : (h + 1) * 128],
                )

        for ch in range(2):
            jj = 2 * k + ch
            for h in range(2):
                for sh in range(2):
                    nc.tensor.matmul(
                        ps[h][:, sh * 512 : (sh + 1) * 512],
                        wT_s[:, jj, h * 128 : (h + 1) * 128],
                        rhsT[:, ch, sh * 512 : (sh + 1) * 512],
                        start=(jj == 0),
                        stop=(jj == 17),
                    )

    # software pipeline: keep one gather in flight ahead of the blend
    vts = {0: gather(0)}
    for k in range(K):
        if k + 1 < K:
            vts[k + 1] = gather(k + 1)
        process(k, vts.pop(k))

    for h in range(2):
        nc.scalar.copy(osb[:, h], ps[h][:])
        nc.sync.dma_start(out[:, h], osb[:, h])

    ctx.close()


# ---------------- host-side prep ----------------

def _host_maps(offset):
    """offset [N, 18, H, W] f32 -> (ridx int16 [N,K,S], cw f32 [N,K,4,S]).

    Gather window start (ys, xs) = clip(floor(p), 0, 62); row = ys*64+xs.
    Patch slot order [TL, BL, TR, BR] = [(ys,xs),(ys+1,xs),(ys,xs+1),(ys+1,xs+1)].
    Weights fold in corner validity and slot selection, matching the
    reference bilinear with zero padding exactly (f32 arithmetic).
    """
    off = offset.reshape(N, K, 2, S).astype(np.float32)
    ky, kx = np.meshgrid(np.arange(KH), np.arange(KW), indexing="ij")
    yy, xx = np.meshgrid(np.arange(H), np.arange(W), indexing="ij")
    base_y = (yy.reshape(1, S) - 1 + ky.reshape(K, 1)).astype(np.float32)
    base_x = (xx.reshape(1, S) - 1 + kx.reshape(K, 1)).astype(np.float32)

    py = base_y[None] + off[:, :, 0]
    px = base_x[None] + off[:, :, 1]

    def axis_slots(p):
        f = np.floor(p)
        l = (p - f).astype(np.float32)
        h = (np.float32(1.0) - l).astype(np.float32)
        fi = f.astype(np.int64)
        cs = np.clip(fi, 0, 62)
        v0 = ((fi >= 0) & (fi < 64)).astype(np.float32)
        v1 = ((fi + 1 >= 0) & (fi + 1 < 64)).astype(np.float32)
        w0 = h * v0  # corner fi
        w1 = l * v1  # corner fi+1
        wA = w0 * (fi == cs) + w1 * (fi + 1 == cs)          # slot at cs
        wB = w0 * (fi == cs + 1) + w1 * (fi + 1 == cs + 1)  # slot at cs+1
        return cs, wA.astype(np.float32), wB.astype(np.float32)

    ys, wyT, wyB = axis_slots(py)
    xs, wxL, wxR = axis_slots(px)
    ridx = (ys * 64 + xs).astype(np.int16)
    cw = np.stack([wyT * wxL, wyB * wxL, wyT * wxR, wyB * wxR], axis=2)
    return ridx, cw.astype(np.float32)


def _expand_image(x):
    """x [N, CIN, H, W] f32 -> xi_ov [N, 4096, 1024] bf16 patch rows."""
    bf = ml_dtypes.bfloat16
    x_cl = np.ascontiguousarray(
        x.reshape(N, CIN, S).transpose(0, 2, 1)
    )  # [n, s, c]
    pad = np.zeros((N, S + 65, CIN), np.float32)
    pad[:, :S] = x_cl
    r = np.arange(S)
    xi = np.concatenate(
        [pad[:, r], pad[:, r + 64], pad[:, r + 1], pad[:, r + 65]], axis=2
    )
    return xi.astype(bf)


def core_inputs(x, offset, weight):
    """Full inputs (np f32) -> list of 8 per-core input dicts."""
    bf = ml_dtypes.bfloat16
    x = np.asarray(x, np.float32)
    offset = np.asarray(offset, np.float32)
    weight = np.asarray(weight, np.float32)

    xi = _expand_image(x)
    ridx, cw = _host_maps(offset)

    wk = weight.reshape(COUT, CIN, K)
    wT = np.ascontiguousarray(
        wk.transpose(2, 1, 0).reshape(K * CIN, COUT)
    ).astype(bf)

    ins = []
    for core in range(8):
        n, q = core // 4, core % 4
        sl = slice(q * SLOC, (q + 1) * SLOC)

        arr = ridx[n, :, sl]                       # [K, 1024]
        # idx i lives at partition i%16, col i//16; the 16-partition pattern
        # is replicated across all 8 gpsimd cores' partition groups.
        iw = np.tile(arr.reshape(K, 64, 16).transpose(2, 0, 1), (8, 1, 1))
        iw = np.ascontiguousarray(iw).astype(np.int16)

        cwq = cw[n, :, :, sl]                      # [K, 4, 1024]
        cws = (
            cwq.reshape(K, 4, TPC, 128)
            .transpose(3, 1, 0, 2)
            .reshape(128, 4, NT)
        )
        ins.append({
            "xi": xi[n],
            "wT": wT,
            "cwf": np.ascontiguousarray(cws).astype(np.float32),
            "idxs": iw,
        })
    return ins


def assemble(results):
    """list of 8 per-core {'out': [128,2,1024] f32} -> [2,256,64,64] f32."""
    out = np.zeros((N, COUT, S), np.float32)
    for core in range(8):
        n, q = core // 4, core % 4
        o = np.asarray(results[core]["out"])       # [128 o_half, 2 h, 1024 s]
        o = o.transpose(1, 0, 2).reshape(COUT, SLOC)
        out[n, :, q * SLOC : (q + 1) * SLOC] = o
    return out.reshape(N, COUT, H, W)


def declare_io(nc):
    ins = {
        "xi": nc.dram_tensor("xi", [S, 1024], BF16, kind="ExternalInput").ap(),
        "wT": nc.dram_tensor("wT", [K * CIN, COUT], BF16, kind="ExternalInput").ap(),
        "cwf": nc.dram_tensor("cwf", [128, 4, NT], F32, kind="ExternalInput").ap(),
        "idxs": nc.dram_tensor("idxs", [128, K, 64], I16, kind="ExternalInput").ap(),
    }
    outs = {
        "out": nc.dram_tensor("out", [128, 2, SLOC], F32, kind="ExternalOutput").ap(),
    }
    return outs, ins


def build_module():
    from concourse import bacc

    nc = bacc.Bacc("TRN2", target_bir_lowering=False, debug=False, num_devices=8)
    outs, ins = declare_io(nc)
    with tile.TileContext(nc) as tc:
        build_core_kernel(nc, tc, outs, ins)
    nc.compile()
    return nc


_NC_CACHE = []


def kernel(x, offset, weight):
    """Full (unsharded) inputs -> full output, computed on 8 NeuronCores."""
    import time

    from concourse.bass_utils import run_bass_kernel_spmd

    if not _NC_CACHE:
        _NC_CACHE.append(build_module())
    nc = _NC_CACHE[0]
    core_ins = core_inputs(x, offset, weight)
    last = None
    for attempt in range(3):
        try:
            res = run_bass_kernel_spmd(nc, core_ins, core_ids=list(range(8)))
            return assemble(res.results)
        except Exception as e:  # transient device-session failures
            last = e
            time.sleep(2.0 * (attempt + 1))
    raise last


# revision 5
# speedup vs baseline: 1.1060x; 1.1060x over previous
"""Self-contained Trainium2 Bass kernel for deformable conv 2d.

kernel(x, offset, weight) -> out, matching the jax reference:
  x[2,256,64,64] f32, offset[2,18,64,64] f32, weight[256,256,3,3] f32
  -> out[2,256,64,64] f32 (KH=KW=3, stride=1, pad=1, dil=1, DG=1).

Runs SPMD on 8 NeuronCores, data-parallel: core = (batch, spatial quarter).

Device pipeline per tap k (host precomputes gather rows + blend weights):
  1. one batched dma_gather: 1024 rows of the pre-expanded patch image
     xi_ov[4096, 1024] (row r = 2x2 patch [TL, BL, TR, BR] x 256ch bf16)
     -> vt [128 samples, 8 slots, 1024]
  2. bilinear blend split across engines (per slot, per-partition scalars):
     ACT: sA = TL*w0, sB = BL*w1; GPSIMD: sC = TR*w2;
     DVE: tD = BR*w3 + sA (stt), then two [128, 2048] adds -> cr
  3. 16 HWDGE xbar transposes [128s,128c] -> rhsT [128c, 2, 1024s]
  4. PE matmuls accumulate over 18 (tap, cin-chunk) into PSUM [2][128, 1024]
"""

import sys

for _p in ("/opt/trn_rl_repo",):
    if _p not in sys.path:
        sys.path.insert(0, _p)


import numpy as np
import ml_dtypes

import concourse.bass as bass
import concourse.mybir as mybir
import concourse.tile as tile

F32 = mybir.dt.float32
BF16 = mybir.dt.bfloat16
I16 = mybir.dt.int16

N, CIN, H, W = 2, 256, 64, 64
COUT = 256
KH = KW = 3
K = KH * KW
S = H * W            # 4096 positions per batch
SLOC = S // 4        # 1024 per core
TPC = 8              # slots per tap (SLOC/128)
NT = K * TPC         # 72 (slot index j = k*8 + t)

AluOp = mybir.AluOpType


def build_core_kernel(nc, tc, outs, ins):
    """Emit the per-core kernel. ins/outs are dicts of DRAM APs."""
    from contextlib import ExitStack

    xi = ins["xi"]          # [4096, 1024] bf16 patch rows
    wT = ins["wT"]          # [2304, 256] bf16 lhsT
    cwf = ins["cwf"]        # [128, 4, 72] f32 blend weights
    idxs = ins["idxs"]      # [128, 9, 64] int16 gather rows
    out = outs["out"]       # [128, 2, 1024] f32

    ctx = ExitStack()
    sp = ctx.enter_context(tc.tile_pool(name="static", bufs=1))
    vtp = ctx.enter_context(tc.tile_pool(name="vt", bufs=2))
    mp = ctx.enter_context(tc.tile_pool(name="mults", bufs=2))
    crp = ctx.enter_context(tc.tile_pool(name="cr", bufs=2))
    rp = ctx.enter_context(tc.tile_pool(name="rhsT", bufs=2))
    pp = ctx.enter_context(tc.tile_pool(name="psum", bufs=1, space="PSUM"))

    v = nc.vector

    # static loads; idxs first (gates the gathers)
    idxs_s = sp.tile([128, K, 64], I16, name="idxs_s")
    nc.sync.dma_start(idxs_s[:], idxs)
    cwf_s = sp.tile([128, 4, NT], F32, name="cwf_s")
    nc.sync.dma_start(cwf_s[:], cwf)
    wT_s = sp.tile([128, 18, 256], BF16, name="wT_s")
    nc.sync.dma_start(wT_s[:], wT.rearrange("(j p) o -> p j o", p=128))

    ps = [pp.tile([128, SLOC], F32, name=f"psum{h}") for h in range(2)]
    osb = sp.tile([128, 2, SLOC], F32, name="osb")

    def gather(k):
        vt = vtp.tile([128, TPC, 1024], BF16, name="vt")
        nc.gpsimd.dma_gather(
            out_ap=vt[:],
            in_ap=xi,
            idxs_ap=idxs_s[:, k, :],
            num_idxs=SLOC,
            num_idxs_reg=SLOC,
            elem_size=1024,
        )
        return vt

    def process(k, vt):
        sA = mp.tile([128, TPC, 256], BF16, name="sA")
        sB = mp.tile([128, TPC, 256], BF16, name="sB")
        ta = crp.tile([128, TPC, 256], BF16, name="ta")
        tb = crp.tile([128, TPC, 256], BF16, name="tb")
        cr = crp.tile([128, TPC, 256], BF16, name="cr")

        for t in range(TPC):
            j = k * TPC + t
            nc.scalar.mul(sA[:, t], vt[:, t, 0:256], cwf_s[:, 0, j : j + 1])
            nc.scalar.mul(sB[:, t], vt[:, t, 256:512], cwf_s[:, 1, j : j + 1])
        for t in range(TPC):
            j = k * TPC + t
            v.scalar_tensor_tensor(
                ta[:, t], vt[:, t, 512:768], cwf_s[:, 2, j : j + 1], sA[:, t],
                AluOp.mult, AluOp.add,
            )
            v.scalar_tensor_tensor(
                tb[:, t], vt[:, t, 768:1024], cwf_s[:, 3, j : j + 1], sB[:, t],
                AluOp.mult, AluOp.add,
            )
            v.tensor_tensor(cr[:, t], ta[:, t], tb[:, t], AluOp.add)

        rhsT = rp.tile([128, 2, SLOC], BF16, name="rhsT")
        for t in range(TPC):
            for h in range(2):
                nc.sync.dma_start_transpose(
                    rhsT[:, h, t * 128 : (t + 1) * 128],
                    cr[:, t, h * 128 : (h + 1) * 128],
                )

        for ch in range(2):
            jj = 2 * k + ch
            for h in range(2):
                for sh in range(2):
                    nc.tensor.matmul(
                        ps[h][:, sh * 512 : (sh + 1) * 512],
                        wT_s[:, jj, h * 128 : (h + 1) * 128],
                        rhsT[:, ch, sh * 512 : (sh + 1) * 512],
                        start=(jj == 0),
                        stop=(jj == 17),
                    )

    # software pipeline: keep one gather in flight ahead of the blend
    vts = {0: gather(0)}
    for k in range(K):
        if k + 1 < K:
            vts[k + 1] = gather(k + 1)
        process(k, vts.pop(k))

    for h in range(2):
        nc.scalar.copy(osb[:, h], ps[h][:])
        nc.sync.dma_start(out[:, h], osb[:, h])

    ctx.close()


# ---------------- host-side prep ----------------

def _host_maps(offset):
    """offset [N, 18, H, W] f32 -> (ridx int16 [N,K,S], cw f32 [N,K,4,S]).

    Gather window start (ys, xs) = clip(floor(p), 0, 62); row = ys*64+xs.
    Patch slot order [TL, BL, TR, BR] = [(ys,xs),(ys+1,xs),(ys,xs+1),(ys+1,xs+1)].
    Weights fold in corner validity and slot selection, matching the
    reference bilinear with zero padding exactly (f32 arithmetic).
    """
    off = offset.reshape(N, K, 2, S).astype(np.float32)
    ky, kx = np.meshgrid(np.arange(KH), np.arange(KW), indexing="ij")
    yy, xx = np.meshgrid(np.arange(H), np.arange(W), indexing="ij")
    base_y = (yy.reshape(1, S) - 1 + ky.reshape(K, 1)).astype(np.float32)
    base_x = (xx.reshape(1, S) - 1 + kx.reshape(K, 1)).astype(np.float32)

    py = base_y[None] + off[:, :, 0]
    px = base_x[None] + off[:, :, 1]

    def axis_slots(p):
        f = np.floor(p)
        l = (p - f).astype(np.float32)
        h = (np.float32(1.0) - l).astype(np.float32)
        fi = f.astype(np.int64)
        cs = np.clip(fi, 0, 62)
        v0 = ((fi >= 0) & (fi < 64)).astype(np.float32)
        v1 = ((fi + 1 >= 0) & (fi + 1 < 64)).astype(np.float32)
        w0 = h * v0  # corner fi
        w1 = l * v1  # corner fi+1
        wA = w0 * (fi == cs) + w1 * (fi + 1 == cs)          # slot at cs
        wB = w0 * (fi == cs + 1) + w1 * (fi + 1 == cs + 1)  # slot at cs+1
        return cs, wA.astype(np.float32), wB.astype(np.float32)

    ys, wyT, wyB = axis_slots(py)
    xs, wxL, wxR = axis_slots(px)
    ridx = (ys * 64 + xs).astype(np.int16)
    cw = np.stack([wyT * wxL, wyB * wxL, wyT * wxR, wyB * wxR], axis=2)
    return ridx, cw.astype(np.float32)


def _expand_image(x):
    """x [N, CIN, H, W] f32 -> xi_ov [N, 4096, 1024] bf16 patch rows."""
    bf = ml_dtypes.bfloat16
    x_cl = np.ascontiguousarray(
        x.reshape(N, CIN, S).transpose(0, 2, 1)
    )  # [n, s, c]
    pad = np.zeros((N, S + 65, CIN), np.float32)
    pad[:, :S] = x_cl
    r = np.arange(S)
    xi = np.concatenate(
        [pad[:, r], pad[:, r + 64], pad[:, r + 1], pad[:, r + 65]], axis=2
    )
    return xi.astype(bf)


def core_inputs(x, offset, weight):
    """Full inputs (np f32) -> list of 8 per-core input dicts."""
    bf = ml_dtypes.bfloat16
    x = np.asarray(x, np.float32)
    offset = np.asarray(offset, np.float32)
    weight = np.asarray(weight, np.float32)

    xi = _expand_image(x)
    ridx, cw = _host_maps(offset)

    wk = weight.reshape(COUT, CIN, K)
    wT = np.ascontiguousarray(
        wk.transpose(2, 1, 0).reshape(K * CIN, COUT)
    ).astype(bf)

    ins = []
    for core in range(8):
        n, q = core // 4, core % 4
        sl = slice(q * SLOC, (q + 1) * SLOC)

        arr = ridx[n, :, sl]                       # [K, 1024]
        # idx i lives at partition i%16, col i//16; the 16-partition pattern
        # is replicated across all 8 gpsimd cores' partition groups.
        iw = np.tile(arr.reshape(K, 64, 16).transpose(2, 0, 1), (8, 1, 1))
        iw = np.ascontiguousarray(iw).astype(np.int16)

        cwq = cw[n, :, :, sl]                      # [K, 4, 1024]
        cws = (
            cwq.reshape(K, 4, TPC, 128)
            .transpose(3, 1, 0, 2)
            .reshape(128, 4, NT)
        )
        ins.append({
            "xi": xi[n],
            "wT": wT,
            "cwf": np.ascontiguousarray(cws).astype(np.float32),
            "idxs": iw,
        })
    return ins


def assemble(results):
    """list of 8 per-core {'out': [128,2,1024] f32} -> [2,256,64,64] f32."""
    out = np.zeros((N, COUT, S), np.float32)
    for core in range(8):
        n, q = core // 4, core % 4
        o = np.asarray(results[core]["out"])       # [128 o_half, 2 h, 1024 s]
        o = o.transpose(1, 0, 2).reshape(COUT, SLOC)
        out[n, :, q * SLOC : (q + 1) * SLOC] = o
    return out.reshape(N, COUT, H, W)


def declare_io(nc):
    ins = {
        "xi": nc.dram_tensor("xi", [S, 1024], BF16, kind="ExternalInput").ap(),
        "wT": nc.dram_tensor("wT", [K * CIN, COUT], BF16, kind="ExternalInput").ap(),
        "cwf": nc.dram_tensor("cwf", [128, 4, NT], F32, kind="ExternalInput").ap(),
        "idxs": nc.dram_tensor("idxs", [128, K, 64], I16, kind="ExternalInput").ap(),
    }
    outs = {
        "out": nc.dram_tensor("out", [128, 2, SLOC], F32, kind="ExternalOutput").ap(),
    }
    return outs, ins


def build_module():
    from concourse import bacc

    nc = bacc.Bacc("TRN2", target_bir_lowering=False, debug=False, num_devices=8)
    outs, ins = declare_io(nc)
    with tile.TileContext(nc) as tc:
        build_core_kernel(nc, tc, outs, ins)
    nc.compile()
    return nc


_NC_CACHE = []


def kernel(x, offset, weight):
    """Full (unsharded) inputs -> full output, computed on 8 NeuronCores."""
    import time

    from concourse.bass_utils import run_bass_kernel_spmd

    if not _NC_CACHE:
        _NC_CACHE.append(build_module())
    nc = _NC_CACHE[0]
    core_ins = core_inputs(x, offset, weight)
    last = None
    for attempt in range(3):
        try:
            res = run_bass_kernel_spmd(nc, core_ins, core_ids=list(range(8)))
            return assemble(res.results)
        except Exception as e:  # transient device-session failures
            last = e
            time.sleep(2.0 * (attempt + 1))
    raise last


# revision 6
# speedup vs baseline: 1.8531x; 1.6755x over previous
"""Self-contained Trainium2 Bass kernel for deformable conv 2d.

kernel(x, offset, weight) -> out, matching the jax reference:
  x[2,256,64,64] f32, offset[2,18,64,64] f32, weight[256,256,3,3] f32
  -> out[2,256,64,64] f32 (KH=KW=3, stride=1, pad=1, dil=1, DG=1).

Runs SPMD on 8 NeuronCores, data-parallel: core = (batch, spatial quarter).

Device pipeline per tap k (host precomputes gather rows + blend weights):
  1. one batched dma_gather: 1024 rows of the pre-expanded patch image
     xi_ov[4096, 1024] (row r = 2x2 patch [TL, BL, TR, BR] x 256ch bf16)
     -> vt [128 samples, 8 slots, 1024]
  2. bilinear blend split across engines (per slot, per-partition scalars):
     ACT: sA = TL*w0, sB = BL*w1; GPSIMD: sC = TR*w2;
     DVE: tD = BR*w3 + sA (stt), then two [128, 2048] adds -> cr
  3. 16 HWDGE xbar transposes [128s,128c] -> rhsT [128c, 2, 1024s]
  4. PE matmuls accumulate over 18 (tap, cin-chunk) into PSUM [2][128, 1024]
"""

import sys

for _p in ("/opt/trn_rl_repo",):
    if _p not in sys.path:
        sys.path.insert(0, _p)


import numpy as np
import ml_dtypes

import concourse.bass as bass
import concourse.mybir as mybir
import concourse.tile as tile

F32 = mybir.dt.float32
BF16 = mybir.dt.bfloat16
I16 = mybir.dt.int16

N, CIN, H, W = 2, 256, 64, 64
COUT = 256
KH = KW = 3
K = KH * KW
S = H * W            # 4096 positions per batch
SLOC = S // 4        # 1024 per core
TPC = 8              # slots per tap (SLOC/128)
NT = K * TPC         # 72 (slot index j = k*8 + t)

AluOp = mybir.AluOpType


def build_core_kernel(nc, tc, outs, ins):
    """Emit the per-core kernel. ins/outs are dicts of DRAM APs."""
    from contextlib import ExitStack

    xi = ins["xi"]          # [4096, 1024] bf16 patch rows
    wT = ins["wT"]          # [2304, 256] bf16 lhsT
    cwf = ins["cwf"]        # [128, 4, 72] f32 blend weights
    idxs = ins["idxs"]      # [128, 9, 64] int16 gather rows
    out = outs["out"]       # [128, 2, 1024] f32

    ctx = ExitStack()
    sp = ctx.enter_context(tc.tile_pool(name="static", bufs=1))
    vtp = ctx.enter_context(tc.tile_pool(name="vt", bufs=2))
    mp = ctx.enter_context(tc.tile_pool(name="mults", bufs=2))
    crp = ctx.enter_context(tc.tile_pool(name="cr", bufs=2))
    rp = ctx.enter_context(tc.tile_pool(name="rhsT", bufs=2))
    pp = ctx.enter_context(tc.tile_pool(name="psum", bufs=1, space="PSUM"))

    v = nc.vector

    # static loads; idxs first (gates the gathers)
    idxs_s = sp.tile([128, K, 64], I16, name="idxs_s")
    nc.sync.dma_start(idxs_s[:], idxs)
    cwf_s = sp.tile([128, 4, NT], F32, name="cwf_s")
    nc.sync.dma_start(cwf_s[:], cwf)
    wT_s = sp.tile([128, 18, 256], BF16, name="wT_s")
    nc.sync.dma_start(wT_s[:], wT.rearrange("(j p) o -> p j o", p=128))

    ps = [pp.tile([128, SLOC], F32, name=f"psum{h}") for h in range(2)]
    osb = sp.tile([128, 2, SLOC], F32, name="osb")

    def gather(k):
        vt = vtp.tile([128, TPC, 1024], BF16, name="vt")
        nc.gpsimd.dma_gather(
            out_ap=vt[:],
            in_ap=xi,
            idxs_ap=idxs_s[:, k, :],
            num_idxs=SLOC,
            num_idxs_reg=SLOC,
            elem_size=1024,
        )
        return vt

    def process(k, vt):
        sA = mp.tile([128, TPC, 256], BF16, name="sA")
        sB = mp.tile([128, TPC, 256], BF16, name="sB")
        ta = crp.tile([128, TPC, 256], BF16, name="ta")
        tb = crp.tile([128, TPC, 256], BF16, name="tb")
        cr = crp.tile([128, 2, TPC, 128], BF16, name="cr")  # half-major

        for t in range(TPC):
            j = k * TPC + t
            nc.scalar.mul(sA[:, t], vt[:, t, 0:256], cwf_s[:, 0, j : j + 1])
            nc.scalar.mul(sB[:, t], vt[:, t, 256:512], cwf_s[:, 1, j : j + 1])
        for t in range(TPC):
            j = k * TPC + t
            v.scalar_tensor_tensor(
                ta[:, t], vt[:, t, 512:768], cwf_s[:, 2, j : j + 1], sA[:, t],
                AluOp.mult, AluOp.add,
            )
            v.scalar_tensor_tensor(
                tb[:, t], vt[:, t, 768:1024], cwf_s[:, 3, j : j + 1], sB[:, t],
                AluOp.mult, AluOp.add,
            )
            v.tensor_tensor(cr[:, :, t, :], ta[:, t], tb[:, t], AluOp.add)

        rhsT = rp.tile([128, 2, SLOC], BF16, name="rhsT")
        for h in range(2):
            # batched xbar: out[c, t, s] = cr[s, h, t, c] for all 8 slots
            nc.sync.dma_start_transpose(
                rhsT[:, h].rearrange("p (t c) -> p t c", t=TPC),
                cr[:, h],
            )

        for ch in range(2):
            jj = 2 * k + ch
            for h in range(2):
                for sh in range(2):
                    nc.tensor.matmul(
                        ps[h][:, sh * 512 : (sh + 1) * 512],
                        wT_s[:, jj, h * 128 : (h + 1) * 128],
                        rhsT[:, ch, sh * 512 : (sh + 1) * 512],
                        start=(jj == 0),
                        stop=(jj == 17),
                    )

    # software pipeline: keep one gather in flight ahead of the blend
    vts = {0: gather(0)}
    for k in range(K):
        if k + 1 < K:
            vts[k + 1] = gather(k + 1)
        process(k, vts.pop(k))

    for h in range(2):
        nc.scalar.copy(osb[:, h], ps[h][:])
        nc.sync.dma_start(out[:, h], osb[:, h])

    ctx.close()


# ---------------- host-side prep ----------------

def _host_maps(offset):
    """offset [N, 18, H, W] f32 -> (ridx int16 [N,K,S], cw f32 [N,K,4,S]).

    Gather window start (ys, xs) = clip(floor(p), 0, 62); row = ys*64+xs.
    Patch slot order [TL, BL, TR, BR] = [(ys,xs),(ys+1,xs),(ys,xs+1),(ys+1,xs+1)].
    Weights fold in corner validity and slot selection, matching the
    reference bilinear with zero padding exactly (f32 arithmetic).
    """
    off = offset.reshape(N, K, 2, S).astype(np.float32)
    ky, kx = np.meshgrid(np.arange(KH), np.arange(KW), indexing="ij")
    yy, xx = np.meshgrid(np.arange(H), np.arange(W), indexing="ij")
    base_y = (yy.reshape(1, S) - 1 + ky.reshape(K, 1)).astype(np.float32)
    base_x = (xx.reshape(1, S) - 1 + kx.reshape(K, 1)).astype(np.float32)

    py = base_y[None] + off[:, :, 0]
    px = base_x[None] + off[:, :, 1]

    def axis_slots(p):
        f = np.floor(p)
        l = (p - f).astype(np.float32)
        h = (np.float32(1.0) - l).astype(np.float32)
        fi = f.astype(np.int64)
        cs = np.clip(fi, 0, 62)
        v0 = ((fi >= 0) & (fi < 64)).astype(np.float32)
        v1 = ((fi + 1 >= 0) & (fi + 1 < 64)).astype(np.float32)
        w0 = h * v0  # corner fi
        w1 = l * v1  # corner fi+1
        wA = w0 * (fi == cs) + w1 * (fi + 1 == cs)          # slot at cs
        wB = w0 * (fi == cs + 1) + w1 * (fi + 1 == cs + 1)  # slot at cs+1
        return cs, wA.astype(np.float32), wB.astype(np.float32)

    ys, wyT, wyB = axis_slots(py)
    xs, wxL, wxR = axis_slots(px)
    ridx = (ys * 64 + xs).astype(np.int16)
    cw = np.stack([wyT * wxL, wyB * wxL, wyT * wxR, wyB * wxR], axis=2)
    return ridx, cw.astype(np.float32)


def _expand_image(x):
    """x [N, CIN, H, W] f32 -> xi_ov [N, 4096, 1024] bf16 patch rows."""
    bf = ml_dtypes.bfloat16
    x_cl = np.ascontiguousarray(
        x.reshape(N, CIN, S).transpose(0, 2, 1)
    )  # [n, s, c]
    pad = np.zeros((N, S + 65, CIN), np.float32)
    pad[:, :S] = x_cl
    r = np.arange(S)
    xi = np.concatenate(
        [pad[:, r], pad[:, r + 64], pad[:, r + 1], pad[:, r + 65]], axis=2
    )
    return xi.astype(bf)


def core_inputs(x, offset, weight):
    """Full inputs (np f32) -> list of 8 per-core input dicts."""
    bf = ml_dtypes.bfloat16
    x = np.asarray(x, np.float32)
    offset = np.asarray(offset, np.float32)
    weight = np.asarray(weight, np.float32)

    xi = _expand_image(x)
    ridx, cw = _host_maps(offset)

    wk = weight.reshape(COUT, CIN, K)
    wT = np.ascontiguousarray(
        wk.transpose(2, 1, 0).reshape(K * CIN, COUT)
    ).astype(bf)

    ins = []
    for core in range(8):
        n, q = core // 4, core % 4
        sl = slice(q * SLOC, (q + 1) * SLOC)

        arr = ridx[n, :, sl]                       # [K, 1024]
        # idx i lives at partition i%16, col i//16; the 16-partition pattern
        # is replicated across all 8 gpsimd cores' partition groups.
        iw = np.tile(arr.reshape(K, 64, 16).transpose(2, 0, 1), (8, 1, 1))
        iw = np.ascontiguousarray(iw).astype(np.int16)

        cwq = cw[n, :, :, sl]                      # [K, 4, 1024]
        cws = (
            cwq.reshape(K, 4, TPC, 128)
            .transpose(3, 1, 0, 2)
            .reshape(128, 4, NT)
        )
        ins.append({
            "xi": xi[n],
            "wT": wT,
            "cwf": np.ascontiguousarray(cws).astype(np.float32),
            "idxs": iw,
        })
    return ins


def assemble(results):
    """list of 8 per-core {'out': [128,2,1024] f32} -> [2,256,64,64] f32."""
    out = np.zeros((N, COUT, S), np.float32)
    for core in range(8):
        n, q = core // 4, core % 4
        o = np.asarray(results[core]["out"])       # [128 o_half, 2 h, 1024 s]
        o = o.transpose(1, 0, 2).reshape(COUT, SLOC)
        out[n, :, q * SLOC : (q + 1) * SLOC] = o
    return out.reshape(N, COUT, H, W)


def declare_io(nc):
    ins = {
        "xi": nc.dram_tensor("xi", [S, 1024], BF16, kind="ExternalInput").ap(),
        "wT": nc.dram_tensor("wT", [K * CIN, COUT], BF16, kind="ExternalInput").ap(),
        "cwf": nc.dram_tensor("cwf", [128, 4, NT], F32, kind="ExternalInput").ap(),
        "idxs": nc.dram_tensor("idxs", [128, K, 64], I16, kind="ExternalInput").ap(),
    }
    outs = {
        "out": nc.dram_tensor("out", [128, 2, SLOC], F32, kind="ExternalOutput").ap(),
    }
    return outs, ins


def build_module():
    from concourse import bacc

    nc = bacc.Bacc("TRN2", target_bir_lowering=False, debug=False, num_devices=8)
    outs, ins = declare_io(nc)
    with tile.TileContext(nc) as tc:
        build_core_kernel(nc, tc, outs, ins)
    nc.compile()
    return nc


_NC_CACHE = []


def kernel(x, offset, weight):
    """Full (unsharded) inputs -> full output, computed on 8 NeuronCores."""
    import time

    from concourse.bass_utils import run_bass_kernel_spmd

    if not _NC_CACHE:
        _NC_CACHE.append(build_module())
    nc = _NC_CACHE[0]
    core_ins = core_inputs(x, offset, weight)
    last = None
    for attempt in range(3):
        try:
            res = run_bass_kernel_spmd(nc, core_ins, core_ids=list(range(8)))
            return assemble(res.results)
        except Exception as e:  # transient device-session failures
            last = e
            time.sleep(2.0 * (attempt + 1))
    raise last


# revision 11
# speedup vs baseline: 1.9021x; 1.0264x over previous
"""Self-contained Trainium2 Bass kernel for deformable conv 2d.

kernel(x, offset, weight) -> out, matching the jax reference:
  x[2,256,64,64] f32, offset[2,18,64,64] f32, weight[256,256,3,3] f32
  -> out[2,256,64,64] f32 (KH=KW=3, stride=1, pad=1, dil=1, DG=1).

Runs SPMD on 8 NeuronCores, data-parallel: core = (batch, spatial quarter).

Device pipeline per tap k (host precomputes gather rows + blend weights):
  1. one batched dma_gather: 1024 rows of the pre-expanded patch image
     xi_ov[4096, 1024] (row r = 2x2 patch [TL, BL, TR, BR] x 256ch bf16)
     -> vt [128 samples, 8 slots, 1024]
  2. bilinear blend split across engines (per slot, per-partition scalars):
     ACT: sA = TL*w0, sB = BL*w1; GPSIMD: sC = TR*w2;
     DVE: tD = BR*w3 + sA (stt), then two [128, 2048] adds -> cr
  3. 16 HWDGE xbar transposes [128s,128c] -> rhsT [128c, 2, 1024s]
  4. PE matmuls accumulate over 18 (tap, cin-chunk) into PSUM [2][128, 1024]
"""

import sys

for _p in ("/opt/trn_rl_repo",):
    if _p not in sys.path:
        sys.path.insert(0, _p)


import numpy as np
import ml_dtypes

import concourse.bass as bass
import concourse.mybir as mybir
import concourse.tile as tile

F32 = mybir.dt.float32
BF16 = mybir.dt.bfloat16
I16 = mybir.dt.int16

N, CIN, H, W = 2, 256, 64, 64
COUT = 256
KH = KW = 3
K = KH * KW
S = H * W            # 4096 positions per batch
SLOC = S // 4        # 1024 per core
TPC = 8              # slots per tap (SLOC/128)
NT = K * TPC         # 72 (slot index j = k*8 + t)

AluOp = mybir.AluOpType


def build_core_kernel(nc, tc, outs, ins):
    """Emit the per-core kernel. ins/outs are dicts of DRAM APs."""
    from contextlib import ExitStack

    xi = ins["xi"]          # [4096, 1024] bf16 patch rows
    wT = ins["wT"]          # [2304, 256] bf16 lhsT
    cwf = ins["cwf"]        # [128, 4, 72] f32 blend weights
    idxs = ins["idxs"]      # [128, 9, 64] int16 gather rows
    out = outs["out"]       # [128, 2, 1024] f32

    ctx = ExitStack()
    sp = ctx.enter_context(tc.tile_pool(name="static", bufs=1))
    vtp = ctx.enter_context(tc.tile_pool(name="vt", bufs=3))
    mp = ctx.enter_context(tc.tile_pool(name="mults", bufs=3))
    crp = ctx.enter_context(tc.tile_pool(name="cr", bufs=3))
    rp = ctx.enter_context(tc.tile_pool(name="rhsT", bufs=3))
    pp = ctx.enter_context(tc.tile_pool(name="psum", bufs=1, space="PSUM"))

    v = nc.vector

    # static loads; idxs first (gates the gathers)
    idxs_s = sp.tile([128, K, 64], I16, name="idxs_s")
    nc.sync.dma_start(idxs_s[:], idxs)
    cwf_s = sp.tile([128, 4, NT], F32, name="cwf_s")
    nc.sync.dma_start(cwf_s[:], cwf)
    wT_s = sp.tile([128, 18, 256], BF16, name="wT_s")
    nc.sync.dma_start(wT_s[:], wT.rearrange("(j p) o -> p j o", p=128))

    ps = [pp.tile([128, SLOC], F32, name=f"psum{h}") for h in range(2)]
    osb = sp.tile([128, 2, SLOC], F32, name="osb")

    def gather(k):
        vt = vtp.tile([128, TPC, 1024], BF16, name="vt")
        nc.gpsimd.dma_gather(
            out_ap=vt[:],
            in_ap=xi,
            idxs_ap=idxs_s[:, k, :],
            num_idxs=SLOC,
            num_idxs_reg=SLOC,
            elem_size=1024,
        )
        return vt

    def process(k, vt, next_gather=None, vts=None):
        sA = mp.tile([128, TPC, 256], BF16, name="sA")
        sB = mp.tile([128, TPC, 256], BF16, name="sB")
        ta = crp.tile([128, TPC, 256], BF16, name="ta")
        tb = crp.tile([128, TPC, 256], BF16, name="tb")
        cr = crp.tile([128, 2, TPC, 128], BF16, name="cr")  # half-major

        for t in range(TPC):
            j = k * TPC + t
            nc.scalar.mul(sA[:, t], vt[:, t, 0:256], cwf_s[:, 0, j : j + 1])
            nc.scalar.mul(sB[:, t], vt[:, t, 256:512], cwf_s[:, 1, j : j + 1])
        for t in range(TPC):
            j = k * TPC + t
            v.scalar_tensor_tensor(
                ta[:, t], vt[:, t, 512:768], cwf_s[:, 2, j : j + 1], sA[:, t],
                AluOp.mult, AluOp.add,
            )
            v.scalar_tensor_tensor(
                tb[:, t], vt[:, t, 768:1024], cwf_s[:, 3, j : j + 1], sB[:, t],
                AluOp.mult, AluOp.add,
            )
            v.tensor_tensor(cr[:, :, t, :], ta[:, t], tb[:, t], AluOp.add)

        rhsT = rp.tile([128, 2, SLOC], BF16, name="rhsT")
        for h in range(2):
            # batched xbar: out[c, t, s] = cr[s, h, t, c] for all 8 slots
            nc.sync.dma_start_transpose(
                rhsT[:, h].rearrange("p (t c) -> p t c", t=TPC),
                cr[:, h],
            )

        if next_gather is not None:
            vts[k + 2] = next_gather()

        for ch in range(2):
            jj = 2 * k + ch
            for h in range(2):
                for sh in range(2):
                    nc.tensor.matmul(
                        ps[h][:, sh * 512 : (sh + 1) * 512],
                        wT_s[:, jj, h * 128 : (h + 1) * 128],
                        rhsT[:, ch, sh * 512 : (sh + 1) * 512],
                        start=(jj == 0),
                        stop=(jj == 17),
                    )

    # software pipeline; emit each tap's transposes before the next gather so
    # the scheduler's transpose-vs-DMA serialization flushes little work
    vts = {0: gather(0), 1: gather(1)}
    for k in range(K):
        nxt = (lambda: gather(k + 2)) if k + 2 < K else None
        process(k, vts.pop(k), nxt, vts)

    for h in range(2):
        nc.scalar.copy(osb[:, h], ps[h][:])
        nc.sync.dma_start(out[:, h], osb[:, h])

    ctx.close()


# ---------------- host-side prep ----------------

def _host_maps(offset):
    """offset [N, 18, H, W] f32 -> (ridx int16 [N,K,S], cw f32 [N,K,4,S]).

    Gather window start (ys, xs) = clip(floor(p), 0, 62); row = ys*64+xs.
    Patch slot order [TL, BL, TR, BR] = [(ys,xs),(ys+1,xs),(ys,xs+1),(ys+1,xs+1)].
    Weights fold in corner validity and slot selection, matching the
    reference bilinear with zero padding exactly (f32 arithmetic).
    """
    off = offset.reshape(N, K, 2, S).astype(np.float32)
    ky, kx = np.meshgrid(np.arange(KH), np.arange(KW), indexing="ij")
    yy, xx = np.meshgrid(np.arange(H), np.arange(W), indexing="ij")
    base_y = (yy.reshape(1, S) - 1 + ky.reshape(K, 1)).astype(np.float32)
    base_x = (xx.reshape(1, S) - 1 + kx.reshape(K, 1)).astype(np.float32)

    py = base_y[None] + off[:, :, 0]
    px = base_x[None] + off[:, :, 1]

    def axis_slots(p):
        f = np.floor(p)
        l = (p - f).astype(np.float32)
        h = (np.float32(1.0) - l).astype(np.float32)
        fi = f.astype(np.int64)
        cs = np.clip(fi, 0, 62)
        v0 = ((fi >= 0) & (fi < 64)).astype(np.float32)
        v1 = ((fi + 1 >= 0) & (fi + 1 < 64)).astype(np.float32)
        w0 = h * v0  # corner fi
        w1 = l * v1  # corner fi+1
        wA = w0 * (fi == cs) + w1 * (fi + 1 == cs)          # slot at cs
        wB = w0 * (fi == cs + 1) + w1 * (fi + 1 == cs + 1)  # slot at cs+1
        return cs, wA.astype(np.float32), wB.astype(np.float32)

    ys, wyT, wyB = axis_slots(py)
    xs, wxL, wxR = axis_slots(px)
    ridx = (ys * 64 + xs).astype(np.int16)
    cw = np.stack([wyT * wxL, wyB * wxL, wyT * wxR, wyB * wxR], axis=2)
    return ridx, cw.astype(np.float32)


def _expand_image(x):
    """x [N, CIN, H, W] f32 -> xi_ov [N, 4096, 1024] bf16 patch rows."""
    bf = ml_dtypes.bfloat16
    x_cl = np.ascontiguousarray(
        x.reshape(N, CIN, S).transpose(0, 2, 1)
    )  # [n, s, c]
    pad = np.zeros((N, S + 65, CIN), np.float32)
    pad[:, :S] = x_cl
    r = np.arange(S)
    xi = np.concatenate(
        [pad[:, r], pad[:, r + 64], pad[:, r + 1], pad[:, r + 65]], axis=2
    )
    return xi.astype(bf)


def core_inputs(x, offset, weight):
    """Full inputs (np f32) -> list of 8 per-core input dicts."""
    bf = ml_dtypes.bfloat16
    x = np.asarray(x, np.float32)
    offset = np.asarray(offset, np.float32)
    weight = np.asarray(weight, np.float32)

    xi = _expand_image(x)
    ridx, cw = _host_maps(offset)

    wk = weight.reshape(COUT, CIN, K)
    wT = np.ascontiguousarray(
        wk.transpose(2, 1, 0).reshape(K * CIN, COUT)
    ).astype(bf)

    ins = []
    for core in range(8):
        n, q = core // 4, core % 4
        sl = slice(q * SLOC, (q + 1) * SLOC)

        arr = ridx[n, :, sl]                       # [K, 1024]
        # idx i lives at partition i%16, col i//16; the 16-partition pattern
        # is replicated across all 8 gpsimd cores' partition groups.
        iw = np.tile(arr.reshape(K, 64, 16).transpose(2, 0, 1), (8, 1, 1))
        iw = np.ascontiguousarray(iw).astype(np.int16)

        cwq = cw[n, :, :, sl]                      # [K, 4, 1024]
        cws = (
            cwq.reshape(K, 4, TPC, 128)
            .transpose(3, 1, 0, 2)
            .reshape(128, 4, NT)
        )
        ins.append({
            "xi": xi[n],
            "wT": wT,
            "cwf": np.ascontiguousarray(cws).astype(np.float32),
            "idxs": iw,
        })
    return ins


def assemble(results):
    """list of 8 per-core {'out': [128,2,1024] f32} -> [2,256,64,64] f32."""
    out = np.zeros((N, COUT, S), np.float32)
    for core in range(8):
        n, q = core // 4, core % 4
        o = np.asarray(results[core]["out"])       # [128 o_half, 2 h, 1024 s]
        o = o.transpose(1, 0, 2).reshape(COUT, SLOC)
        out[n, :, q * SLOC : (q + 1) * SLOC] = o
    return out.reshape(N, COUT, H, W)


def declare_io(nc):
    ins = {
        "xi": nc.dram_tensor("xi", [S, 1024], BF16, kind="ExternalInput").ap(),
        "wT": nc.dram_tensor("wT", [K * CIN, COUT], BF16, kind="ExternalInput").ap(),
        "cwf": nc.dram_tensor("cwf", [128, 4, NT], F32, kind="ExternalInput").ap(),
        "idxs": nc.dram_tensor("idxs", [128, K, 64], I16, kind="ExternalInput").ap(),
    }
    outs = {
        "out": nc.dram_tensor("out", [128, 2, SLOC], F32, kind="ExternalOutput").ap(),
    }
    return outs, ins


def build_module():
    from concourse import bacc

    nc = bacc.Bacc("TRN2", target_bir_lowering=False, debug=False, num_devices=8)
    outs, ins = declare_io(nc)
    with tile.TileContext(nc) as tc:
        build_core_kernel(nc, tc, outs, ins)
    nc.compile()
    return nc


_NC_CACHE = []


def kernel(x, offset, weight):
    """Full (unsharded) inputs -> full output, computed on 8 NeuronCores."""
    import time

    from concourse.bass_utils import run_bass_kernel_spmd

    if not _NC_CACHE:
        _NC_CACHE.append(build_module())
    nc = _NC_CACHE[0]
    core_ins = core_inputs(x, offset, weight)
    last = None
    for attempt in range(3):
        try:
            res = run_bass_kernel_spmd(nc, core_ins, core_ids=list(range(8)))
            return assemble(res.results)
        except Exception as e:  # transient device-session failures
            last = e
            time.sleep(2.0 * (attempt + 1))
    raise last


# revision 14
# speedup vs baseline: 1.9888x; 1.0456x over previous
"""Self-contained Trainium2 Bass kernel for deformable conv 2d.

kernel(x, offset, weight) -> out, matching the jax reference:
  x[2,256,64,64] f32, offset[2,18,64,64] f32, weight[256,256,3,3] f32
  -> out[2,256,64,64] f32 (KH=KW=3, stride=1, pad=1, dil=1, DG=1).

Runs SPMD on 8 NeuronCores, data-parallel: core = (batch, spatial quarter).

Device pipeline per tap k (host precomputes gather rows + blend weights):
  1. one batched dma_gather: 1024 rows of the pre-expanded patch image
     xi_ov[4096, 1024] (row r = 2x2 patch [TL, BL, TR, BR] x 256ch bf16)
     -> vt [128 samples, 8 slots, 1024]
  2. bilinear blend split across engines (per slot, per-partition scalars):
     ACT: sA = TL*w0, sB = BL*w1; GPSIMD: sC = TR*w2;
     DVE: tD = BR*w3 + sA (stt), then two [128, 2048] adds -> cr
  3. 16 HWDGE xbar transposes [128s,128c] -> rhsT [128c, 2, 1024s]
  4. PE matmuls accumulate over 18 (tap, cin-chunk) into PSUM [2][128, 1024]
"""

import sys

for _p in ("/opt/trn_rl_repo",):
    if _p not in sys.path:
        sys.path.insert(0, _p)


import numpy as np
import ml_dtypes

import concourse.bass as bass
import concourse.mybir as mybir
import concourse.tile as tile

F32 = mybir.dt.float32
BF16 = mybir.dt.bfloat16
I16 = mybir.dt.int16

N, CIN, H, W = 2, 256, 64, 64
COUT = 256
KH = KW = 3
K = KH * KW
S = H * W            # 4096 positions per batch
SLOC = S // 4        # 1024 per core
TPC = 8              # slots per tap (SLOC/128)
NT = K * TPC         # 72 (slot index j = k*8 + t)

AluOp = mybir.AluOpType


def build_core_kernel(nc, tc, outs, ins):
    """Emit the per-core kernel. ins/outs are dicts of DRAM APs."""
    from contextlib import ExitStack

    xi = ins["xi"]          # [4096, 1024] bf16 patch rows
    wT = ins["wT"]          # [2304, 256] bf16 lhsT
    cwf = ins["cwf"]        # [128, 4, 72] f32 blend weights
    idxs = ins["idxs"]      # [128, 9, 64] int16 gather rows
    out = outs["out"]       # [128, 2, 1024] f32

    ctx = ExitStack()
    sp = ctx.enter_context(tc.tile_pool(name="static", bufs=1))
    vtp = ctx.enter_context(tc.tile_pool(name="vt", bufs=4))
    mp = ctx.enter_context(tc.tile_pool(name="mults", bufs=3))
    crp = ctx.enter_context(tc.tile_pool(name="cr", bufs=4))
    rp = ctx.enter_context(tc.tile_pool(name="rhsT", bufs=4))
    pp = ctx.enter_context(tc.tile_pool(name="psum", bufs=1, space="PSUM"))

    v = nc.vector

    # static loads; idxs first (gates the gathers)
    idxs_s = sp.tile([128, K + 1, 64], I16, name="idxs_s")
    nc.sync.dma_start(idxs_s[:], idxs)
    cwf_s = sp.tile([128, 4, NT], F32, name="cwf_s")
    nc.sync.dma_start(cwf_s[:], cwf)
    wT_s = sp.tile([128, 18, 256], BF16, name="wT_s")
    nc.sync.dma_start(wT_s[:], wT.rearrange("(j p) o -> p j o", p=128))

    ps = [pp.tile([128, SLOC], F32, name=f"psum{h}") for h in range(2)]
    osb = sp.tile([128, 2, SLOC], F32, name="osb")

    def gather(k):
        vt = vtp.tile([128, TPC, 1024], BF16, name="vt")
        nc.gpsimd.dma_gather(
            out_ap=vt[:],
            in_ap=xi,
            idxs_ap=idxs_s[:, k, :],
            num_idxs=SLOC,
            num_idxs_reg=SLOC,
            elem_size=1024,
        )
        return vt

    def process(k, vt):
        sA = mp.tile([128, TPC, 256], BF16, name="sA")
        sB = mp.tile([128, TPC, 256], BF16, name="sB")
        ta = crp.tile([128, TPC, 256], BF16, name="ta")
        tb = crp.tile([128, TPC, 256], BF16, name="tb")
        cr = crp.tile([128, 2, TPC, 128], BF16, name="cr")  # half-major

        for t in range(TPC):
            j = k * TPC + t
            nc.scalar.mul(sA[:, t], vt[:, t, 0:256], cwf_s[:, 0, j : j + 1])
            nc.scalar.mul(sB[:, t], vt[:, t, 256:512], cwf_s[:, 1, j : j + 1])
        for t in range(TPC):
            j = k * TPC + t
            v.scalar_tensor_tensor(
                ta[:, t], vt[:, t, 512:768], cwf_s[:, 2, j : j + 1], sA[:, t],
                AluOp.mult, AluOp.add,
            )
            v.scalar_tensor_tensor(
                tb[:, t], vt[:, t, 768:1024], cwf_s[:, 3, j : j + 1], sB[:, t],
                AluOp.mult, AluOp.add,
            )
            v.tensor_tensor(cr[:, :, t, :], ta[:, t], tb[:, t], AluOp.add)

        return cr

    def transpose_tap(cr):
        rhsT = rp.tile([128, 2, SLOC], BF16, name="rhsT")
        for h in range(2):
            # batched xbar: out[c, t, s] = cr[s, h, t, c] for all 8 slots
            nc.sync.dma_start_transpose(
                rhsT[:, h].rearrange("p (t c) -> p t c", t=TPC),
                cr[:, h],
            )
        return rhsT

    def matmul_tap(k, rhsT):
        for ch in range(2):
            jj = 2 * k + ch
            for h in range(2):
                for sh in range(2):
                    nc.tensor.matmul(
                        ps[h][:, sh * 512 : (sh + 1) * 512],
                        wT_s[:, jj, h * 128 : (h + 1) * 128],
                        rhsT[:, ch, sh * 512 : (sh + 1) * 512],
                        start=(jj == 0),
                        stop=(jj == 17),
                    )

    # Pipeline in groups of 3 taps: blends stream with gathers; each group's
    # 6 xbar transposes form ONE scheduler flush point (transposes serialize
    # against all in-flight DMA), then the group's matmuls run on PE.
    vts = {0: gather(0), 1: gather(1), 2: gather(2)}
    for grp in range(3):
        crs = []
        for k in range(grp * 3, grp * 3 + 3):
            if k + 3 < K:
                vts[k + 3] = gather(k + 3)
            crs.append(process(k, vts.pop(k)))
        rhsTs = [transpose_tap(cr) for cr in crs]
        for i, k in enumerate(range(grp * 3, grp * 3 + 3)):
            matmul_tap(k, rhsTs[i])

    for h in range(2):
        nc.scalar.copy(osb[:, h], ps[h][:])
        nc.sync.dma_start(out[:, h], osb[:, h])

    ctx.close()


# ---------------- host-side prep ----------------

def _host_maps(offset):
    """offset [N, 18, H, W] f32 -> (ridx int16 [N,K,S], cw f32 [N,K,4,S]).

    Gather window start (ys, xs) = clip(floor(p), 0, 62); row = ys*64+xs.
    Patch slot order [TL, BL, TR, BR] = [(ys,xs),(ys+1,xs),(ys,xs+1),(ys+1,xs+1)].
    Weights fold in corner validity and slot selection, matching the
    reference bilinear with zero padding exactly (f32 arithmetic).
    """
    off = offset.reshape(N, K, 2, S).astype(np.float32)
    ky, kx = np.meshgrid(np.arange(KH), np.arange(KW), indexing="ij")
    yy, xx = np.meshgrid(np.arange(H), np.arange(W), indexing="ij")
    base_y = (yy.reshape(1, S) - 1 + ky.reshape(K, 1)).astype(np.float32)
    base_x = (xx.reshape(1, S) - 1 + kx.reshape(K, 1)).astype(np.float32)

    py = base_y[None] + off[:, :, 0]
    px = base_x[None] + off[:, :, 1]

    def axis_slots(p):
        f = np.floor(p)
        l = (p - f).astype(np.float32)
        h = (np.float32(1.0) - l).astype(np.float32)
        fi = f.astype(np.int64)
        cs = np.clip(fi, 0, 62)
        v0 = ((fi >= 0) & (fi < 64)).astype(np.float32)
        v1 = ((fi + 1 >= 0) & (fi + 1 < 64)).astype(np.float32)
        w0 = h * v0  # corner fi
        w1 = l * v1  # corner fi+1
        wA = w0 * (fi == cs) + w1 * (fi + 1 == cs)          # slot at cs
        wB = w0 * (fi == cs + 1) + w1 * (fi + 1 == cs + 1)  # slot at cs+1
        return cs, wA.astype(np.float32), wB.astype(np.float32)

    ys, wyT, wyB = axis_slots(py)
    xs, wxL, wxR = axis_slots(px)
    ridx = (ys * 64 + xs).astype(np.int16)
    cw = np.stack([wyT * wxL, wyB * wxL, wyT * wxR, wyB * wxR], axis=2)
    return ridx, cw.astype(np.float32)


def _expand_image(x):
    """x [N, CIN, H, W] f32 -> xi_ov [N, 4096, 1024] bf16 patch rows."""
    bf = ml_dtypes.bfloat16
    x_cl = np.ascontiguousarray(
        x.reshape(N, CIN, S).transpose(0, 2, 1)
    )  # [n, s, c]
    pad = np.zeros((N, S + 65, CIN), np.float32)
    pad[:, :S] = x_cl
    r = np.arange(S)
    xi = np.concatenate(
        [pad[:, r], pad[:, r + 64], pad[:, r + 1], pad[:, r + 65]], axis=2
    )
    return xi.astype(bf)


def core_inputs(x, offset, weight):
    """Full inputs (np f32) -> list of 8 per-core input dicts."""
    bf = ml_dtypes.bfloat16
    x = np.asarray(x, np.float32)
    offset = np.asarray(offset, np.float32)
    weight = np.asarray(weight, np.float32)

    xi = _expand_image(x)
    ridx, cw = _host_maps(offset)

    wk = weight.reshape(COUT, CIN, K)
    wT = np.ascontiguousarray(
        wk.transpose(2, 1, 0).reshape(K * CIN, COUT)
    ).astype(bf)

    ins = []
    for core in range(8):
        n, q = core // 4, core % 4
        sl = slice(q * SLOC, (q + 1) * SLOC)

        arr = np.concatenate([ridx[n, :, sl], np.arange(SLOC, dtype=np.int16)[None]])
        # idx i lives at partition i%16, col i//16; the 16-partition pattern
        # is replicated across all 8 gpsimd cores' partition groups.
        iw = np.tile(arr.reshape(K + 1, 64, 16).transpose(2, 0, 1), (8, 1, 1))
        iw = np.ascontiguousarray(iw).astype(np.int16)

        cwq = cw[n, :, :, sl]                      # [K, 4, 1024]
        cws = (
            cwq.reshape(K, 4, TPC, 128)
            .transpose(3, 1, 0, 2)
            .reshape(128, 4, NT)
        )
        ins.append({
            "xi": xi[n],
            "wT": wT,
            "cwf": np.ascontiguousarray(cws).astype(np.float32),
            "idxs": iw,
        })
    return ins


def assemble(results):
    """list of 8 per-core {'out': [128,2,1024] f32} -> [2,256,64,64] f32."""
    out = np.zeros((N, COUT, S), np.float32)
    for core in range(8):
        n, q = core // 4, core % 4
        o = np.asarray(results[core]["out"])       # [128 o_half, 2 h, 1024 s]
        o = o.transpose(1, 0, 2).reshape(COUT, SLOC)
        out[n, :, q * SLOC : (q + 1) * SLOC] = o
    return out.reshape(N, COUT, H, W)


def declare_io(nc):
    ins = {
        "xi": nc.dram_tensor("xi", [S, 1024], BF16, kind="ExternalInput").ap(),
        "wT": nc.dram_tensor("wT", [K * CIN, COUT], BF16, kind="ExternalInput").ap(),
        "cwf": nc.dram_tensor("cwf", [128, 4, NT], F32, kind="ExternalInput").ap(),
        "idxs": nc.dram_tensor("idxs", [128, K + 1, 64], I16, kind="ExternalInput").ap(),
    }
    outs = {
        "out": nc.dram_tensor("out", [128, 2, SLOC], F32, kind="ExternalOutput").ap(),
    }
    return outs, ins


def build_module():
    from concourse import bacc

    nc = bacc.Bacc("TRN2", target_bir_lowering=False, debug=False, num_devices=8)
    outs, ins = declare_io(nc)
    with tile.TileContext(nc) as tc:
        build_core_kernel(nc, tc, outs, ins)
    nc.compile()
    return nc


_NC_CACHE = []


def kernel(x, offset, weight):
    """Full (unsharded) inputs -> full output, computed on 8 NeuronCores."""
    import time

    from concourse.bass_utils import run_bass_kernel_spmd

    if not _NC_CACHE:
        _NC_CACHE.append(build_module())
    nc = _NC_CACHE[0]
    core_ins = core_inputs(x, offset, weight)
    last = None
    for attempt in range(3):
        try:
            res = run_bass_kernel_spmd(nc, core_ins, core_ids=list(range(8)))
            out = assemble(res.results)
            if np.isfinite(out).all():
                return out
            last = RuntimeError("non-finite output")  # rare HW flake: retry
        except Exception as e:  # transient device-session failures
            last = e
            time.sleep(2.0 * (attempt + 1))
    raise last


# revision 16
# speedup vs baseline: 3.2086x; 1.6133x over previous
"""Self-contained Trainium2 Bass kernel for deformable conv 2d.

kernel(x, offset, weight) -> out, matching the jax reference:
  x[2,256,64,64] f32, offset[2,18,64,64] f32, weight[256,256,3,3] f32
  -> out[2,256,64,64] f32 (KH=KW=3, stride=1, pad=1, dil=1, DG=1).

Runs SPMD on 8 NeuronCores, data-parallel: core = (batch, spatial quarter).

Device pipeline per tap k (host precomputes gather rows + blend weights):
  1. one batched dma_gather: 1024 rows of the pre-expanded patch image
     xi_ov[4096, 1024] (row r = 2x2 patch [TL, BL, TR, BR] x 256ch bf16)
     -> vt [128 samples, 8 slots, 1024]
  2. bilinear blend split across engines (per slot, per-partition scalars):
     ACT: sA = TL*w0, sB = BL*w1; GPSIMD: sC = TR*w2;
     DVE: tD = BR*w3 + sA (stt), then two [128, 2048] adds -> cr
  3. 16 HWDGE xbar transposes [128s,128c] -> rhsT [128c, 2, 1024s]
  4. PE matmuls accumulate over 18 (tap, cin-chunk) into PSUM [2][128, 1024]
"""

import sys

for _p in ("/opt/trn_rl_repo",):
    if _p not in sys.path:
        sys.path.insert(0, _p)


import numpy as np
import ml_dtypes

import concourse.bass as bass
import concourse.mybir as mybir
import concourse.tile as tile

F32 = mybir.dt.float32
BF16 = mybir.dt.bfloat16
I16 = mybir.dt.int16

N, CIN, H, W = 2, 256, 64, 64
COUT = 256
KH = KW = 3
K = KH * KW
S = H * W            # 4096 positions per batch
SLOC = S // 4        # 1024 per core
TPC = 8              # slots per tap (SLOC/128)
NT = K * TPC         # 72 (slot index j = k*8 + t)

AluOp = mybir.AluOpType


def build_core_kernel(nc, tc, outs, ins):
    """Emit the per-core kernel. ins/outs are dicts of DRAM APs."""
    from contextlib import ExitStack

    xi = ins["xi"]          # [4096, 1024] bf16 patch rows
    ident = ins["ident"]    # [128, 128] bf16 identity
    wT = ins["wT"]          # [2304, 256] bf16 lhsT
    cwf = ins["cwf"]        # [128, 4, 72] f32 blend weights
    idxs = ins["idxs"]      # [128, 9, 64] int16 gather rows
    out = outs["out"]       # [128, 2, 1024] f32

    ctx = ExitStack()
    sp = ctx.enter_context(tc.tile_pool(name="static", bufs=1))
    vtp = ctx.enter_context(tc.tile_pool(name="vt", bufs=4))
    mp = ctx.enter_context(tc.tile_pool(name="mults", bufs=3))
    crp = ctx.enter_context(tc.tile_pool(name="cr", bufs=4))
    rp = ctx.enter_context(tc.tile_pool(name="rhsT", bufs=4))
    pp = ctx.enter_context(tc.tile_pool(name="psum", bufs=1, space="PSUM"))
    tp = ctx.enter_context(tc.tile_pool(name="tpsum", bufs=4, space="PSUM"))

    v = nc.vector

    # static loads; idxs first (gates the gathers)
    idxs_s = sp.tile([128, K + 1, 64], I16, name="idxs_s")
    nc.sync.dma_start(idxs_s[:], idxs)
    cwf_s = sp.tile([128, 4, NT], F32, name="cwf_s")
    nc.sync.dma_start(cwf_s[:], cwf)
    wT_s = sp.tile([128, 18, 256], BF16, name="wT_s")
    nc.sync.dma_start(wT_s[:], wT.rearrange("(j p) o -> p j o", p=128))
    id_s = sp.tile([128, 128], BF16, name="id_s")
    nc.sync.dma_start(id_s[:], ident)

    ps = [pp.tile([128, SLOC], F32, name=f"psum{h}") for h in range(2)]
    osb = sp.tile([128, 2, SLOC], F32, name="osb")

    def gather(k):
        vt = vtp.tile([128, TPC, 1024], BF16, name="vt")
        nc.gpsimd.dma_gather(
            out_ap=vt[:],
            in_ap=xi,
            idxs_ap=idxs_s[:, k, :],
            num_idxs=SLOC,
            num_idxs_reg=SLOC,
            elem_size=1024,
        )
        return vt

    def process(k, vt):
        sA = mp.tile([128, TPC, 256], BF16, name="sA")
        sB = mp.tile([128, TPC, 256], BF16, name="sB")
        ta = crp.tile([128, TPC, 256], BF16, name="ta")
        tb = crp.tile([128, TPC, 256], BF16, name="tb")
        cr = crp.tile([128, TPC, 256], BF16, name="cr")

        for t in range(TPC):
            j = k * TPC + t
            nc.scalar.mul(sA[:, t], vt[:, t, 0:256], cwf_s[:, 0, j : j + 1])
            nc.scalar.mul(sB[:, t], vt[:, t, 256:512], cwf_s[:, 1, j : j + 1])
        for t in range(TPC):
            j = k * TPC + t
            v.scalar_tensor_tensor(
                ta[:, t], vt[:, t, 512:768], cwf_s[:, 2, j : j + 1], sA[:, t],
                AluOp.mult, AluOp.add,
            )
            v.scalar_tensor_tensor(
                tb[:, t], vt[:, t, 768:1024], cwf_s[:, 3, j : j + 1], sB[:, t],
                AluOp.mult, AluOp.add,
            )
            v.tensor_tensor(cr[:, t], ta[:, t], tb[:, t], AluOp.add)

        # transpose on the PE (identity matmuls, bf16 PSUM), evac split
        # between ACT and DVE to balance engine load
        rhsT = rp.tile([128, 2, SLOC], BF16, name="rhsT")
        for g in range(2):
            pt = tp.tile([128, 2, 4, 128], BF16, name="tpsum", space="PSUM")
            for q in range(4):
                t = g * 4 + q
                for h in range(2):
                    nc.tensor.matmul(
                        pt[:, h, q, :], cr[:, t, h * 128 : (h + 1) * 128],
                        id_s[:], is_transpose=True, start=True, stop=True,
                    )
            dst = rhsT[:, :, g * 512 : (g + 1) * 512].rearrange(
                "p a (c b) -> p a c b", c=4
            )
            if g == 0:
                nc.scalar.copy(dst, pt[:])
            else:
                v.tensor_copy(dst, pt[:])

        for ch in range(2):
            jj = 2 * k + ch
            for h in range(2):
                for sh in range(2):
                    nc.tensor.matmul(
                        ps[h][:, sh * 512 : (sh + 1) * 512],
                        wT_s[:, jj, h * 128 : (h + 1) * 128],
                        rhsT[:, ch, sh * 512 : (sh + 1) * 512],
                        start=(jj == 0),
                        stop=(jj == 17),
                    )

    vts = {0: gather(0), 1: gather(1)}
    for k in range(K):
        if k + 2 < K:
            vts[k + 2] = gather(k + 2)
        process(k, vts.pop(k))

    for h in range(2):
        nc.scalar.copy(osb[:, h], ps[h][:])
        nc.sync.dma_start(out[:, h], osb[:, h])

    ctx.close()


# ---------------- host-side prep ----------------

def _host_maps(offset):
    """offset [N, 18, H, W] f32 -> (ridx int16 [N,K,S], cw f32 [N,K,4,S]).

    Gather window start (ys, xs) = clip(floor(p), 0, 62); row = ys*64+xs.
    Patch slot order [TL, BL, TR, BR] = [(ys,xs),(ys+1,xs),(ys,xs+1),(ys+1,xs+1)].
    Weights fold in corner validity and slot selection, matching the
    reference bilinear with zero padding exactly (f32 arithmetic).
    """
    off = offset.reshape(N, K, 2, S).astype(np.float32)
    ky, kx = np.meshgrid(np.arange(KH), np.arange(KW), indexing="ij")
    yy, xx = np.meshgrid(np.arange(H), np.arange(W), indexing="ij")
    base_y = (yy.reshape(1, S) - 1 + ky.reshape(K, 1)).astype(np.float32)
    base_x = (xx.reshape(1, S) - 1 + kx.reshape(K, 1)).astype(np.float32)

    py = base_y[None] + off[:, :, 0]
    px = base_x[None] + off[:, :, 1]

    def axis_slots(p):
        f = np.floor(p)
        l = (p - f).astype(np.float32)
        h = (np.float32(1.0) - l).astype(np.float32)
        fi = f.astype(np.int64)
        cs = np.clip(fi, 0, 62)
        v0 = ((fi >= 0) & (fi < 64)).astype(np.float32)
        v1 = ((fi + 1 >= 0) & (fi + 1 < 64)).astype(np.float32)
        w0 = h * v0  # corner fi
        w1 = l * v1  # corner fi+1
        wA = w0 * (fi == cs) + w1 * (fi + 1 == cs)          # slot at cs
        wB = w0 * (fi == cs + 1) + w1 * (fi + 1 == cs + 1)  # slot at cs+1
        return cs, wA.astype(np.float32), wB.astype(np.float32)

    ys, wyT, wyB = axis_slots(py)
    xs, wxL, wxR = axis_slots(px)
    ridx = (ys * 64 + xs).astype(np.int16)
    cw = np.stack([wyT * wxL, wyB * wxL, wyT * wxR, wyB * wxR], axis=2)
    return ridx, cw.astype(np.float32)


def _expand_image(x):
    """x [N, CIN, H, W] f32 -> xi_ov [N, 4096, 1024] bf16 patch rows."""
    bf = ml_dtypes.bfloat16
    x_cl = np.ascontiguousarray(
        x.reshape(N, CIN, S).transpose(0, 2, 1)
    )  # [n, s, c]
    pad = np.zeros((N, S + 65, CIN), np.float32)
    pad[:, :S] = x_cl
    r = np.arange(S)
    xi = np.concatenate(
        [pad[:, r], pad[:, r + 64], pad[:, r + 1], pad[:, r + 65]], axis=2
    )
    return xi.astype(bf)


def core_inputs(x, offset, weight):
    """Full inputs (np f32) -> list of 8 per-core input dicts."""
    bf = ml_dtypes.bfloat16
    x = np.asarray(x, np.float32)
    offset = np.asarray(offset, np.float32)
    weight = np.asarray(weight, np.float32)

    xi = _expand_image(x)
    ridx, cw = _host_maps(offset)

    wk = weight.reshape(COUT, CIN, K)
    wT = np.ascontiguousarray(
        wk.transpose(2, 1, 0).reshape(K * CIN, COUT)
    ).astype(bf)

    ins = []
    for core in range(8):
        n, q = core // 4, core % 4
        sl = slice(q * SLOC, (q + 1) * SLOC)

        arr = np.concatenate([ridx[n, :, sl], np.arange(SLOC, dtype=np.int16)[None]])
        # idx i lives at partition i%16, col i//16; the 16-partition pattern
        # is replicated across all 8 gpsimd cores' partition groups.
        iw = np.tile(arr.reshape(K + 1, 64, 16).transpose(2, 0, 1), (8, 1, 1))
        iw = np.ascontiguousarray(iw).astype(np.int16)

        cwq = cw[n, :, :, sl]                      # [K, 4, 1024]
        cws = (
            cwq.reshape(K, 4, TPC, 128)
            .transpose(3, 1, 0, 2)
            .reshape(128, 4, NT)
        )
        ins.append({
            "xi": xi[n],
            "wT": wT,
            "cwf": np.ascontiguousarray(cws).astype(np.float32),
            "ident": np.eye(128, dtype=bf),
            "idxs": iw,
        })
    return ins


def assemble(results):
    """list of 8 per-core {'out': [128,2,1024] f32} -> [2,256,64,64] f32."""
    out = np.zeros((N, COUT, S), np.float32)
    for core in range(8):
        n, q = core // 4, core % 4
        o = np.asarray(results[core]["out"])       # [128 o_half, 2 h, 1024 s]
        o = o.transpose(1, 0, 2).reshape(COUT, SLOC)
        out[n, :, q * SLOC : (q + 1) * SLOC] = o
    return out.reshape(N, COUT, H, W)


def declare_io(nc):
    ins = {
        "xi": nc.dram_tensor("xi", [S, 1024], BF16, kind="ExternalInput").ap(),
        "wT": nc.dram_tensor("wT", [K * CIN, COUT], BF16, kind="ExternalInput").ap(),
        "cwf": nc.dram_tensor("cwf", [128, 4, NT], F32, kind="ExternalInput").ap(),
        "idxs": nc.dram_tensor("idxs", [128, K + 1, 64], I16, kind="ExternalInput").ap(),
        "ident": nc.dram_tensor("ident", [128, 128], BF16, kind="ExternalInput").ap(),
    }
    outs = {
        "out": nc.dram_tensor("out", [128, 2, SLOC], F32, kind="ExternalOutput").ap(),
    }
    return outs, ins


def build_module():
    from concourse import bacc

    nc = bacc.Bacc("TRN2", target_bir_lowering=False, debug=False, num_devices=8)
    outs, ins = declare_io(nc)
    with tile.TileContext(nc) as tc:
        build_core_kernel(nc, tc, outs, ins)
    nc.compile()
    return nc


_NC_CACHE = []


def kernel(x, offset, weight):
    """Full (unsharded) inputs -> full output, computed on 8 NeuronCores."""
    import time

    from concourse.bass_utils import run_bass_kernel_spmd

    if not _NC_CACHE:
        _NC_CACHE.append(build_module())
    nc = _NC_CACHE[0]
    core_ins = core_inputs(x, offset, weight)
    last = None
    for attempt in range(3):
        try:
            res = run_bass_kernel_spmd(nc, core_ins, core_ids=list(range(8)))
            out = assemble(res.results)
            if np.isfinite(out).all():
                return out
            last = RuntimeError("non-finite output")  # rare HW flake: retry
        except Exception as e:  # transient device-session failures
            last = e
            time.sleep(2.0 * (attempt + 1))
    raise last
